# revision 1
# baseline (speedup 1.0000x reference)
"""Bahdanau-style attention scoring kernel for 8 TRN2 NeuronCores.

Reference computation (B=128, H=256, N=2048):
    hidden = concat([static, dynamic, broadcast(dec)], axis=1)   # [B, 3H, N]
    scores = tanh(einsum('hk,bkn->bhn', W[0], hidden))           # [B, H, N]
    logits = einsum('h,bhn->bn', v[0,0], scores)                 # [B, N]
    attns  = softmax(logits, axis=-1)[:, None, :]                # [B, 1, N]

Strategy (measured 195-200us on a quiet chip, 215-235us when the chip's
HBM throughput sags to ~320GB/s mid-run -- that mode is environmental,
identical NEFFs measure both ways; earlier versions were 210-290us):

- Data-parallel over batch: 16 batches per core, no collectives. The tiny
  W / v params are replicated (pre-cast to bf16 on host); the broadcast
  decoder term collapses to a per-batch bias c[b] = W_dec @ dec[b]
  (precomputed on host, 0.003% of FLOPs).

- Matmuls run in bf16 at ~227ns per 512-col tile (near the 2.4GHz PE
  roofline). f32 activations are DMA'd raw; the matmul rhs reads the high
  2 bytes of each f32 in SBUF (stride-2 bf16 bitcast view) -- bf16
  truncation for free, no cast pass on any engine. End-to-end rel err
  ~2e-3 vs the 2e-2 gate.

- x loads: the host repacks both sources into xr[b, p, j, n] (j = xs k0,
  xs k1, xd k0, xd k1 -- a pure layout change; each partition's batch
  slice becomes one contiguous 32KB DRAM run), so a batch is TWO flat
  2MB HWDGE DMAs with a single 16KB run per partition = 128 cheap
  descriptors each. That halves sequencer descriptor-gen time (sync busy
  48us -> 29us) and sem traffic vs four 1MB DMAs, and the ring sustains
  ~415-430GB/s. DMA sizing is sensitive: 512KB DMAs drop the ring to
  ~310GB/s, and multi-run-per-partition APs cost ~4x more descriptor-gen
  (4.5-6.4us per 2MB), so j is the only safe split axis. Prefetch is 5
  batches deep; batches 0/1 split j singly (plus columns for b0) so the
  first matmuls start ~11us in. Parameter DMAs go on the idle GPSIMD
  (SWDGE) ring.

- Main loop is nt-outer / kt-inner over 1-bank PSUM units with 6 slots:
  each (nt, m) unit's 4 accumulating matmuls run consecutively and its
  tanh follows immediately, so tanh work, PSUM slot releases, and the sc
  columns the v-matmuls need are produced evenly through the batch. (The
  v1 kernel's 2x2-bank slots stalled the PE ~3.5us/batch on the
  tanh->slot-WAR chain; this shape has ~4us of slack on it.)

- All 64 logits land in ONE PSUM bank: the masked v-matmul for n-tile nt
  writes output partitions [32*nt, 32*nt+32) via an explicit
  tile_position=(0, 32*nt) (the auto-inferred path rejects base 96), with
  vm columns 16..31 zero so the unused partitions accumulate exact zeros
  instead of stale PSUM garbage. v-matmuls run one batch behind the
  mains and are emitted at the HEAD of each batch: their sc inputs are
  already complete, giving the PE ~1.8us of ready work in exactly the
  window where the batch's x tiles and completion receipts are still
  landing (batch-head DMA waits measured 2-4us on congested draws).
  Softmax is then: one exp over the whole [128,512] bank with
  accum_out row sums, a tiny f32 mask-matmul that sums each batch's 4
  quarter-sums across partitions, reciprocal, one [128,512] scale, and 4
  output DMAs -- ~6us of tail.

Built as a bacc.Bacc graph (its compile() pass redistributes multi-sem
waits; raw Bass hits the hardware's one-sync-wait-per-instruction limit).
"""

import sys

if "/opt/trn_rl_repo" not in sys.path:
    sys.path.insert(0, "/opt/trn_rl_repo")

import numpy as np

B, H, N = 128, 256, 2048
NCORES = 8
BPC = B // NCORES  # batches per core
P = 128            # SBUF partitions
KT = 4             # k-tiles over 2H=512 contraction
MT = 2             # m-tiles over H=256 output rows
NS = 512           # n-tile (one PSUM bank of f32)
NT = N // NS       # 4 n-tiles
PREFETCH = 5       # batches of x in flight

_CACHE = {}


def _build():
    import concourse.bacc as bacc
    from concourse import mybir
    from concourse.tile import TileContext

    f32 = mybir.dt.float32
    bf16 = mybir.dt.bfloat16
    Tanh = mybir.ActivationFunctionType.Tanh
    Exp = mybir.ActivationFunctionType.Exp

    nc = bacc.Bacc()
    # xr[b, p, j, n]: host-repacked activations; j = (xs k0, xs k1, xd k0,
    # xd k1), so each partition's batch slice is ONE 32KB contiguous DRAM
    # run and a j-pair DMA is a single 16KB run per partition.
    xr = nc.declare_dram_parameter("xr", [BPC, P, KT, N], f32, isOutput=False)
    # wt[k, h] = W[h, k] for k in [0, 512): rows 0:256 static, 256:512 dynamic
    wt = nc.declare_dram_parameter("wt", [2 * H, H], bf16, isOutput=False)
    # cb[h, b] = sum_k W[h, 512+k] * dec[b, k]  (host-precomputed bias)
    cb = nc.declare_dram_parameter("cb", [H, BPC], f32, isOutput=False)
    # vm[p, b, m, j] = v[m*128 + p] * (j == b); columns 16..31 are zero
    vm = nc.declare_dram_parameter("vm", [P, BPC, MT, 32], bf16, isOutput=False)
    # msk[p, j] = ((p%32) == (j%32)) & ((p%32) < 16): partition-sum matrix
    msk = nc.declare_dram_parameter("msk", [P, P], f32, isOutput=False)
    out = nc.declare_dram_parameter("out", [BPC, N], f32, isOutput=True)

    with (
        TileContext(nc) as tc,
        tc.tile_pool(name="const", bufs=1) as cpool,
        tc.tile_pool(name="xh", bufs=PREFETCH) as hpool,
        tc.tile_pool(name="sc", bufs=2) as spool,
        tc.tile_pool(name="ps", bufs=6, space="PSUM") as ppool,
        tc.tile_pool(name="pl", bufs=1, space="PSUM") as plpool,
        tc.tile_pool(name="pq", bufs=1, space="PSUM") as pqpool,
    ):
        # --- x loads: one [128, 4, 2048] f32 tile per batch, filled by two
        # 2MB HWDGE DMAs (j-pairs; one 16KB contiguous DRAM run per
        # partition => 128 cheap descriptors each). Multi-run-per-partition
        # APs cost ~4x more descriptor-gen, and 512KB DMAs drop the ring to
        # ~310GB/s, so j is the only safe split axis for steady batches;
        # ramp batches split j singly (8KB runs) plus by columns (j-extent
        # 1 keeps it one run per partition).
        xf_tiles = {}

        def issue_x_dmas(bb, jsplit=2, csplit=1):
            xt = hpool.tile([P, KT, N], f32, name=f"xt{bb}", tag="xt")
            step = N // csplit
            for cs0 in range(csplit):
                cs = slice(cs0 * step, (cs0 + 1) * step)
                for j0 in range(0, KT, jsplit):
                    js = slice(j0, j0 + jsplit)
                    nc.sync.dma_start(
                        out=xt[:, js, cs],
                        in_=xr[bb, :, js, cs],
                    )
            xf_tiles[bb] = xt

        # --- replicated parameters on the idle GPSIMD (SWDGE) ring ---
        wt_sb = []
        for kt in range(KT):
            w = cpool.tile([P, H], bf16, name=f"wt{kt}", tag=f"wt{kt}")
            nc.gpsimd.dma_start(out=w[:], in_=wt[kt * P:(kt + 1) * P, :])
            wt_sb.append(w)
        vm_sb = cpool.tile([P, BPC, MT, 32], bf16)
        nc.gpsimd.dma_start(out=vm_sb[:], in_=vm[:])
        # bias laid out [128, m, b]
        c_sb = cpool.tile([P, MT, BPC], f32)
        nc.gpsimd.dma_start(out=c_sb[:], in_=cb[:].rearrange("(m p) b -> p m b", p=P))
        msk_sb = cpool.tile([P, P], f32)
        nc.gpsimd.dma_start(out=msk_sb[:], in_=msk[:])

        issue_x_dmas(0, jsplit=1, csplit=2)
        issue_x_dmas(1, jsplit=1)
        for bb in range(2, PREFETCH):
            issue_x_dmas(bb)

        # logits accumulator: ONE PSUM bank. The masked v-matmul for
        # (batch b, n-tile nt) lands batch b's 512 logits on partition
        # 32*nt + b, accumulating all 16 batches x 2 m-halves per quarter.
        lp = plpool.tile([P, NS], f32)

        sc_hist = {}

        def emit_vmms(vb):
            sc_prev = sc_hist.pop(vb)
            for m in range(MT):
                for nt in range(NT):
                    nc.tensor.matmul(
                        lp[32 * nt:32 * nt + 32, :],
                        lhsT=vm_sb[:, vb, m, :],
                        rhs=sc_prev[:, m, nt * NS:(nt + 1) * NS],
                        start=(vb == 0 and m == 0),
                        stop=(vb == BPC - 1 and m == MT - 1),
                        tile_position=(0, 32 * nt),
                    )

        # --- main loop ---
        for b in range(BPC):
            if b + PREFETCH < BPC:
                issue_x_dmas(b + PREFETCH)
            # v-matmuls of the previous batch go FIRST: their sc inputs are
            # already complete, so they give the PE ~1.8us of ready work
            # exactly in the window where batch b's x tiles (and their
            # completion receipts) are still landing -- the batch-head
            # DMA-wait measured 2-4us/batch on congested draws.
            if b > 0:
                emit_vmms(b - 1)
            xt = xf_tiles.pop(b)
            xh = [xt[:].bitcast(bf16)[:, kt, 1::2] for kt in range(KT)]

            # nt-outer / kt-inner: each (nt, m) PSUM unit's 4 accumulating
            # matmuls run consecutively and its tanh follows immediately,
            # so slot releases (and the sc columns the v-matmuls need) are
            # produced evenly through the batch instead of bunching at
            # m-group ends.
            sc_t = spool.tile([P, MT, N], bf16, tag="sc")
            for nt in range(NT):
                ns = slice(nt * NS, (nt + 1) * NS)
                for m in range(MT):
                    pst = ppool.tile([P, NS], f32, tag="pst", name=f"pst{m}_{nt}")
                    for kt in range(KT):
                        nc.tensor.matmul(
                            pst[:],
                            lhsT=wt_sb[kt][:, m * P:(m + 1) * P],
                            rhs=xh[kt][:, ns],
                            start=(kt == 0),
                            stop=(kt == KT - 1),
                        )
                    nc.scalar.activation(
                        sc_t[:, m, ns], pst[:], Tanh,
                        bias=c_sb[:, m, b:b + 1],
                    )
                    if b == BPC - 1:
                        # last batch: emit each v-matmul right after its
                        # tanh so only ONE vmm (not 8) sits between the
                        # final tanh and the softmax chain (~1.6us tail cut)
                        nc.tensor.matmul(
                            lp[32 * nt:32 * nt + 32, :],
                            lhsT=vm_sb[:, b, m, :],
                            rhs=sc_t[:, m, ns],
                            start=False,
                            stop=(m == MT - 1),
                            tile_position=(0, 32 * nt),
                        )
            if b < BPC - 1:
                sc_hist[b] = sc_t

        # --- softmax over N per batch row (no max-subtraction: |logits| <~ 10)
        # exp of the whole logits bank at once; row sums via accum_out.
        exp_sb = cpool.tile([P, NS], f32)
        psums = cpool.tile([P, 1], f32)
        nc.scalar.activation(exp_sb[:], lp[:], Exp, accum_out=psums[:])
        # sum each batch's 4 quarter-sums across partitions: lpsum[32nt+b]
        # = sum_nt' psums[32nt'+b] for all 4 nt (f32 matmul, 1 column).
        lpsum = pqpool.tile([P, 1], f32)
        nc.tensor.matmul(lpsum[:], lhsT=msk_sb[:], rhs=psums[:], start=True, stop=True)
        rec = cpool.tile([P, 1], f32)
        nc.vector.reciprocal(rec[:], lpsum[:])
        nc.vector.tensor_scalar_mul(exp_sb[:], exp_sb[:], rec[:])
        for nt in range(NT):
            nc.sync.dma_start(
                out=out[:, nt * NS:(nt + 1) * NS],
                in_=exp_sb[32 * nt:32 * nt + BPC, :],
            )

    nc.compile()
    return nc


def _make_in_maps(static_hidden, dynamic_hidden, decoder_hidden, v, W):
    import ml_dtypes

    bf16 = ml_dtypes.bfloat16
    W0 = np.asarray(W, dtype=np.float32)[0]          # [256, 768]
    wt_np = np.ascontiguousarray(W0[:, :2 * H].T.astype(bf16))   # [512, 256]
    vhalf = np.asarray(v, dtype=np.float32)[0, 0].reshape(MT, P)  # [2, 128]
    # vm[p, b, m, j] = v[m*128+p] * (j == b); j in [0, 32), cols 16..31 zero
    eye = np.zeros((BPC, 32), dtype=np.float32)
    eye[np.arange(BPC), np.arange(BPC)] = 1.0
    vm_np = np.ascontiguousarray(
        np.einsum("mp,bj->pbmj", vhalf, eye).astype(bf16)
    )
    # msk[p, j] = ((p%32) == (j%32)) & ((p%32) < 16)
    pp = np.arange(P)
    msk_np = np.ascontiguousarray(
        (((pp[:, None] % 32) == (pp[None, :] % 32)) & ((pp[:, None] % 32) < 16))
        .astype(np.float32)
    )

    sh = np.asarray(static_hidden, dtype=np.float32).reshape(B, 2, P, N)
    dh = np.asarray(dynamic_hidden, dtype=np.float32).reshape(B, 2, P, N)
    # xr[b, p, j, n], j = (xs k0, xs k1, xd k0, xd k1): pure layout repack
    # so each partition's batch slice is one contiguous 32KB DRAM run
    xr_full = np.concatenate(
        (sh.transpose(0, 2, 1, 3), dh.transpose(0, 2, 1, 3)), axis=2
    )                                                # [B, 128, 4, 2048]
    dec = np.asarray(decoder_hidden, dtype=np.float32)
    # cb[h, b] = sum_k W_dec[h, k] dec[b, k], fp32 on host (tiny)
    cb_full = W0[:, 2 * H:] @ dec.T                  # [256, B]

    in_maps = []
    for i in range(NCORES):
        sl = slice(i * BPC, (i + 1) * BPC)
        in_maps.append({
            "xr": np.ascontiguousarray(xr_full[sl]),
            "wt": wt_np,
            "cb": np.ascontiguousarray(cb_full[:, sl]),
            "vm": vm_np,
            "msk": msk_np,
        })
    return in_maps


def kernel(static_hidden, dynamic_hidden, decoder_hidden, v, W):
    from concourse.bass_utils import run_bass_kernel_spmd

    if "nc" not in _CACHE:
        _CACHE["nc"] = _build()
    nc = _CACHE["nc"]

    in_maps = _make_in_maps(static_hidden, dynamic_hidden, decoder_hidden, v, W)
    res = run_bass_kernel_spmd(nc, in_maps, core_ids=list(range(NCORES)))
    out = np.concatenate([r["out"] for r in res.results], axis=0)
    return out.reshape(B, 1, N).astype(np.float32)



# revision 6
# speedup vs baseline: 1.1961x; 1.1961x over previous
"""Bahdanau-style attention scoring kernel for 8 TRN2 NeuronCores.

Reference computation (B=128, H=256, N=2048):
    hidden = concat([static, dynamic, broadcast(dec)], axis=1)   # [B, 3H, N]
    scores = tanh(einsum('hk,bkn->bhn', W[0], hidden))           # [B, H, N]
    logits = einsum('h,bhn->bn', v[0,0], scores)                 # [B, N]
    attns  = softmax(logits, axis=-1)[:, None, :]                # [B, 1, N]

Strategy (measured 195-200us on a quiet chip, 215-235us when the chip's
HBM throughput sags to ~320GB/s mid-run -- that mode is environmental,
identical NEFFs measure both ways; earlier versions were 210-290us):

- Data-parallel over batch: 16 batches per core, no collectives. The tiny
  W / v params are replicated (pre-cast to bf16 on host); the broadcast
  decoder term collapses to a per-batch bias c[b] = W_dec @ dec[b]
  (precomputed on host, 0.003% of FLOPs).

- Matmuls run in bf16 at ~227ns per 512-col tile (near the 2.4GHz PE
  roofline). f32 activations are DMA'd raw; the matmul rhs reads the high
  2 bytes of each f32 in SBUF (stride-2 bf16 bitcast view) -- bf16
  truncation for free, no cast pass on any engine. End-to-end rel err
  ~2e-3 vs the 2e-2 gate.

- x loads: the host repacks both sources into xr[b, p, j, n] (j = xs k0,
  xs k1, xd k0, xd k1 -- a pure layout change; each partition's batch
  slice becomes one contiguous 32KB DRAM run), so a batch is TWO flat
  2MB HWDGE DMAs with a single 16KB run per partition = 128 cheap
  descriptors each. That halves sequencer descriptor-gen time (sync busy
  48us -> 29us) and sem traffic vs four 1MB DMAs, and the ring sustains
  ~415-430GB/s. DMA sizing is sensitive: 512KB DMAs drop the ring to
  ~310GB/s, and multi-run-per-partition APs cost ~4x more descriptor-gen
  (4.5-6.4us per 2MB), so j is the only safe split axis. Prefetch is 5
  batches deep; batches 0/1 split j singly (plus columns for b0) so the
  first matmuls start ~11us in. Parameter DMAs go on the idle GPSIMD
  (SWDGE) ring.

- Main loop is nt-outer / kt-inner over 1-bank PSUM units with 6 slots:
  each (nt, m) unit's 4 accumulating matmuls run consecutively and its
  tanh follows immediately, so tanh work, PSUM slot releases, and the sc
  columns the v-matmuls need are produced evenly through the batch. (The
  v1 kernel's 2x2-bank slots stalled the PE ~3.5us/batch on the
  tanh->slot-WAR chain; this shape has ~4us of slack on it.)

- All 64 logits land in ONE PSUM bank: the masked v-matmul for n-tile nt
  writes output partitions [32*nt, 32*nt+32) via an explicit
  tile_position=(0, 32*nt) (the auto-inferred path rejects base 96), with
  vm columns 16..31 zero so the unused partitions accumulate exact zeros
  instead of stale PSUM garbage. v-matmuls run one batch behind the
  mains and are emitted at the HEAD of each batch: their sc inputs are
  already complete, giving the PE ~1.8us of ready work in exactly the
  window where the batch's x tiles and completion receipts are still
  landing (batch-head DMA waits measured 2-4us on congested draws).
  Softmax is then: one exp over the whole [128,512] bank with
  accum_out row sums, a tiny f32 mask-matmul that sums each batch's 4
  quarter-sums across partitions, reciprocal, one [128,512] scale, and 4
  output DMAs -- ~6us of tail.

Built as a bacc.Bacc graph (its compile() pass redistributes multi-sem
waits; raw Bass hits the hardware's one-sync-wait-per-instruction limit).
"""

import sys

if "/opt/trn_rl_repo" not in sys.path:
    sys.path.insert(0, "/opt/trn_rl_repo")

import numpy as np

B, H, N = 128, 256, 2048
NCORES = 8
BPC = B // NCORES  # batches per core
P = 128            # SBUF partitions
KT = 4             # k-tiles over 2H=512 contraction
MT = 2             # m-tiles over H=256 output rows
NS = 512           # n-tile (one PSUM bank of f32)
NT = N // NS       # 4 n-tiles
PREFETCH = 8       # batches of x in flight (bf16 x halves SBUF per batch)

_CACHE = {}


def _build():
    import concourse.bacc as bacc
    from concourse import mybir
    from concourse.tile import TileContext

    f32 = mybir.dt.float32
    bf16 = mybir.dt.bfloat16
    Tanh = mybir.ActivationFunctionType.Tanh
    Exp = mybir.ActivationFunctionType.Exp

    nc = bacc.Bacc()
    # xr[b, p, j, n]: host-repacked activations, TRUNCATED to bf16 on the
    # host (same numerics as v1's in-SBUF high-2-byte bitcast, but HALF the
    # HBM traffic: 32MB/core instead of 64MB). j = (xs k0, xs k1, xd k0,
    # xd k1), so each partition's batch slice is ONE 16KB contiguous DRAM
    # run and a j-pair DMA is a single 8KB run per partition.
    xr = nc.declare_dram_parameter("xr", [BPC, P, KT, N], bf16, isOutput=False)
    # wt[k, h] = W[h, k] for k in [0, 512): rows 0:256 static, 256:512 dynamic
    wt = nc.declare_dram_parameter("wt", [2 * H, H], bf16, isOutput=False)
    # cb[h, b] = sum_k W[h, 512+k] * dec[b, k]  (host-precomputed bias)
    cb = nc.declare_dram_parameter("cb", [H, BPC], f32, isOutput=False)
    # vm[p, b, m, j] = v[m*128 + p] * (j == b); columns 16..31 are zero
    vm = nc.declare_dram_parameter("vm", [P, BPC, MT, 32], bf16, isOutput=False)
    # msk[p, j] = ((p%32) == (j%32)) & ((p%32) < 16): partition-sum matrix
    msk = nc.declare_dram_parameter("msk", [P, P], f32, isOutput=False)
    out = nc.declare_dram_parameter("out", [BPC, N], f32, isOutput=True)

    with (
        TileContext(nc) as tc,
        tc.tile_pool(name="const", bufs=1) as cpool,
        tc.tile_pool(name="xh", bufs=PREFETCH) as hpool,
        tc.tile_pool(name="sc", bufs=2) as spool,
        tc.tile_pool(name="ps", bufs=6, space="PSUM") as ppool,
        tc.tile_pool(name="pl", bufs=1, space="PSUM") as plpool,
        tc.tile_pool(name="pq", bufs=1, space="PSUM") as pqpool,
    ):
        # --- x loads: one [128, 4, 2048] bf16 tile per batch, filled by two
        # 1MB HWDGE DMAs (j-pairs; one 8KB contiguous DRAM run per
        # partition => 128 cheap descriptors each). Multi-run-per-partition
        # APs cost ~4x more descriptor-gen, so j is the only safe split
        # axis for steady batches; ramp batches split j singly (4KB runs)
        # plus by columns (j-extent 1 keeps it one run per partition).
        xf_tiles = {}

        def issue_x_dmas(bb, jsplit=2, csplit=1):
            xt = hpool.tile([P, KT, N], bf16, name=f"xt{bb}", tag="xt")
            step = N // csplit
            for cs0 in range(csplit):
                cs = slice(cs0 * step, (cs0 + 1) * step)
                for j0 in range(0, KT, jsplit):
                    js = slice(j0, j0 + jsplit)
                    nc.sync.dma_start(
                        out=xt[:, js, cs],
                        in_=xr[bb, :, js, cs],
                    )
            xf_tiles[bb] = xt

        # --- replicated parameters on the idle GPSIMD (SWDGE) ring ---
        wt_sb = []
        for kt in range(KT):
            w = cpool.tile([P, H], bf16, name=f"wt{kt}", tag=f"wt{kt}")
            nc.gpsimd.dma_start(out=w[:], in_=wt[kt * P:(kt + 1) * P, :])
            wt_sb.append(w)
        vm_sb = cpool.tile([P, BPC, MT, 32], bf16)
        nc.gpsimd.dma_start(out=vm_sb[:], in_=vm[:])
        # bias laid out [128, m, b]
        c_sb = cpool.tile([P, MT, BPC], f32)
        nc.gpsimd.dma_start(out=c_sb[:], in_=cb[:].rearrange("(m p) b -> p m b", p=P))
        msk_sb = cpool.tile([P, P], f32)
        nc.gpsimd.dma_start(out=msk_sb[:], in_=msk[:])

        issue_x_dmas(0, jsplit=1, csplit=2)
        issue_x_dmas(1, jsplit=1)
        for bb in range(2, PREFETCH):
            issue_x_dmas(bb)

        # logits accumulator: ONE PSUM bank. The masked v-matmul for
        # (batch b, n-tile nt) lands batch b's 512 logits on partition
        # 32*nt + b, accumulating all 16 batches x 2 m-halves per quarter.
        lp = plpool.tile([P, NS], f32)

        sc_hist = {}

        def emit_vmms(vb):
            sc_prev = sc_hist.pop(vb)
            for m in range(MT):
                for nt in range(NT):
                    nc.tensor.matmul(
                        lp[32 * nt:32 * nt + 32, :],
                        lhsT=vm_sb[:, vb, m, :],
                        rhs=sc_prev[:, m, nt * NS:(nt + 1) * NS],
                        start=(vb == 0 and m == 0),
                        stop=(vb == BPC - 1 and m == MT - 1),
                        tile_position=(0, 32 * nt),
                    )

        # --- main loop ---
        for b in range(BPC):
            if b + PREFETCH < BPC:
                issue_x_dmas(b + PREFETCH)
            # v-matmuls of the previous batch go FIRST: their sc inputs are
            # already complete, so they give the PE ~1.8us of ready work
            # exactly in the window where batch b's x tiles (and their
            # completion receipts) are still landing -- the batch-head
            # DMA-wait measured 2-4us/batch on congested draws.
            if b > 0:
                emit_vmms(b - 1)
            xt = xf_tiles.pop(b)
            xh = [xt[:, kt, :] for kt in range(KT)]

            # nt-outer / kt-inner: each (nt, m) PSUM unit's 4 accumulating
            # matmuls run consecutively and its tanh follows immediately,
            # so slot releases (and the sc columns the v-matmuls need) are
            # produced evenly through the batch instead of bunching at
            # m-group ends.
            sc_t = spool.tile([P, MT, N], bf16, tag="sc")
            for nt in range(NT):
                ns = slice(nt * NS, (nt + 1) * NS)
                for m in range(MT):
                    pst = ppool.tile([P, NS], f32, tag="pst", name=f"pst{m}_{nt}")
                    for kt in range(KT):
                        nc.tensor.matmul(
                            pst[:],
                            lhsT=wt_sb[kt][:, m * P:(m + 1) * P],
                            rhs=xh[kt][:, ns],
                            start=(kt == 0),
                            stop=(kt == KT - 1),
                        )
                    nc.scalar.activation(
                        sc_t[:, m, ns], pst[:], Tanh,
                        bias=c_sb[:, m, b:b + 1],
                    )
                    if b == BPC - 1:
                        # last batch: emit each v-matmul right after its
                        # tanh so only ONE vmm (not 8) sits between the
                        # final tanh and the softmax chain (~1.6us tail cut)
                        nc.tensor.matmul(
                            lp[32 * nt:32 * nt + 32, :],
                            lhsT=vm_sb[:, b, m, :],
                            rhs=sc_t[:, m, ns],
                            start=False,
                            stop=(m == MT - 1),
                            tile_position=(0, 32 * nt),
                        )
            if b < BPC - 1:
                sc_hist[b] = sc_t

        # --- softmax over N per batch row (no max-subtraction: |logits| <~ 10)
        # exp of the whole logits bank at once; row sums via accum_out.
        exp_sb = cpool.tile([P, NS], f32)
        psums = cpool.tile([P, 1], f32)
        nc.scalar.activation(exp_sb[:], lp[:], Exp, accum_out=psums[:])
        # sum each batch's 4 quarter-sums across partitions: lpsum[32nt+b]
        # = sum_nt' psums[32nt'+b] for all 4 nt (f32 matmul, 1 column).
        lpsum = pqpool.tile([P, 1], f32)
        nc.tensor.matmul(lpsum[:], lhsT=msk_sb[:], rhs=psums[:], start=True, stop=True)
        rec = cpool.tile([P, 1], f32)
        nc.vector.reciprocal(rec[:], lpsum[:])
        nc.vector.tensor_scalar_mul(exp_sb[:], exp_sb[:], rec[:])
        for nt in range(NT):
            nc.sync.dma_start(
                out=out[:, nt * NS:(nt + 1) * NS],
                in_=exp_sb[32 * nt:32 * nt + BPC, :],
            )

    nc.compile()
    return nc


def _make_in_maps(static_hidden, dynamic_hidden, decoder_hidden, v, W):
    import ml_dtypes

    bf16 = ml_dtypes.bfloat16
    W0 = np.asarray(W, dtype=np.float32)[0]          # [256, 768]
    wt_np = np.ascontiguousarray(W0[:, :2 * H].T.astype(bf16))   # [512, 256]
    vhalf = np.asarray(v, dtype=np.float32)[0, 0].reshape(MT, P)  # [2, 128]
    # vm[p, b, m, j] = v[m*128+p] * (j == b); j in [0, 32), cols 16..31 zero
    eye = np.zeros((BPC, 32), dtype=np.float32)
    eye[np.arange(BPC), np.arange(BPC)] = 1.0
    vm_np = np.ascontiguousarray(
        np.einsum("mp,bj->pbmj", vhalf, eye).astype(bf16)
    )
    # msk[p, j] = ((p%32) == (j%32)) & ((p%32) < 16)
    pp = np.arange(P)
    msk_np = np.ascontiguousarray(
        (((pp[:, None] % 32) == (pp[None, :] % 32)) & ((pp[:, None] % 32) < 16))
        .astype(np.float32)
    )

    sh = np.asarray(static_hidden, dtype=np.float32)
    dh = np.asarray(dynamic_hidden, dtype=np.float32)
    # Truncate f32 -> bf16 on the host (keep the high 2 bytes of each f32;
    # little-endian so uint16 index 1). Identical numerics to v1's in-SBUF
    # stride-2 bitcast, but the DMA moves half the bytes.
    shu = sh.view(np.uint16).reshape(B, 2, P, N, 2)[..., 1]
    dhu = dh.view(np.uint16).reshape(B, 2, P, N, 2)[..., 1]
    # xr[b, p, j, n], j = (xs k0, xs k1, xd k0, xd k1): layout repack so
    # each partition's batch slice is one contiguous 16KB DRAM run
    xr_full = np.concatenate(
        (shu.transpose(0, 2, 1, 3), dhu.transpose(0, 2, 1, 3)), axis=2
    ).view(bf16)                                     # [B, 128, 4, 2048] bf16
    dec = np.asarray(decoder_hidden, dtype=np.float32)
    # cb[h, b] = sum_k W_dec[h, k] dec[b, k], fp32 on host (tiny)
    cb_full = W0[:, 2 * H:] @ dec.T                  # [256, B]

    in_maps = []
    for i in range(NCORES):
        sl = slice(i * BPC, (i + 1) * BPC)
        in_maps.append({
            "xr": np.ascontiguousarray(xr_full[sl]),
            "wt": wt_np,
            "cb": np.ascontiguousarray(cb_full[:, sl]),
            "vm": vm_np,
            "msk": msk_np,
        })
    return in_maps


def kernel(static_hidden, dynamic_hidden, decoder_hidden, v, W):
    from concourse.bass_utils import run_bass_kernel_spmd

    if "nc" not in _CACHE:
        _CACHE["nc"] = _build()
    nc = _CACHE["nc"]

    in_maps = _make_in_maps(static_hidden, dynamic_hidden, decoder_hidden, v, W)
    res = run_bass_kernel_spmd(nc, in_maps, core_ids=list(range(NCORES)))
    out = np.concatenate([r["out"] for r in res.results], axis=0)
    return out.reshape(B, 1, N).astype(np.float32)



# revision 9
# speedup vs baseline: 1.4828x; 1.2397x over previous
"""Bahdanau-style attention scoring kernel for 8 TRN2 NeuronCores.

Reference computation (B=128, H=256, N=2048):
    hidden = concat([static, dynamic, broadcast(dec)], axis=1)   # [B, 3H, N]
    scores = tanh(einsum('hk,bkn->bhn', W[0], hidden))           # [B, H, N]
    logits = einsum('h,bhn->bn', v[0,0], scores)                 # [B, N]
    attns  = softmax(logits, axis=-1)[:, None, :]                # [B, 1, N]

Strategy (measured 195-200us on a quiet chip, 215-235us when the chip's
HBM throughput sags to ~320GB/s mid-run -- that mode is environmental,
identical NEFFs measure both ways; earlier versions were 210-290us):

- Data-parallel over batch: 16 batches per core, no collectives. The tiny
  W / v params are replicated (pre-cast to bf16 on host); the broadcast
  decoder term collapses to a per-batch bias c[b] = W_dec @ dec[b]
  (precomputed on host, 0.003% of FLOPs).

- Matmuls run in bf16 at ~227ns per 512-col tile (near the 2.4GHz PE
  roofline). f32 activations are DMA'd raw; the matmul rhs reads the high
  2 bytes of each f32 in SBUF (stride-2 bf16 bitcast view) -- bf16
  truncation for free, no cast pass on any engine. End-to-end rel err
  ~2e-3 vs the 2e-2 gate.

- x loads: the host repacks both sources into xr[b, p, j, n] (j = xs k0,
  xs k1, xd k0, xd k1 -- a pure layout change; each partition's batch
  slice becomes one contiguous 32KB DRAM run), so a batch is TWO flat
  2MB HWDGE DMAs with a single 16KB run per partition = 128 cheap
  descriptors each. That halves sequencer descriptor-gen time (sync busy
  48us -> 29us) and sem traffic vs four 1MB DMAs, and the ring sustains
  ~415-430GB/s. DMA sizing is sensitive: 512KB DMAs drop the ring to
  ~310GB/s, and multi-run-per-partition APs cost ~4x more descriptor-gen
  (4.5-6.4us per 2MB), so j is the only safe split axis. Prefetch is 5
  batches deep; batches 0/1 split j singly (plus columns for b0) so the
  first matmuls start ~11us in. Parameter DMAs go on the idle GPSIMD
  (SWDGE) ring.

- Main loop is nt-outer / kt-inner over 1-bank PSUM units with 6 slots:
  each (nt, m) unit's 4 accumulating matmuls run consecutively and its
  tanh follows immediately, so tanh work, PSUM slot releases, and the sc
  columns the v-matmuls need are produced evenly through the batch. (The
  v1 kernel's 2x2-bank slots stalled the PE ~3.5us/batch on the
  tanh->slot-WAR chain; this shape has ~4us of slack on it.)

- All 64 logits land in ONE PSUM bank: the masked v-matmul for n-tile nt
  writes output partitions [32*nt, 32*nt+32) via an explicit
  tile_position=(0, 32*nt) (the auto-inferred path rejects base 96), with
  vm columns 16..31 zero so the unused partitions accumulate exact zeros
  instead of stale PSUM garbage. v-matmuls run one batch behind the
  mains and are emitted at the HEAD of each batch: their sc inputs are
  already complete, giving the PE ~1.8us of ready work in exactly the
  window where the batch's x tiles and completion receipts are still
  landing (batch-head DMA waits measured 2-4us on congested draws).
  Softmax is then: one exp over the whole [128,512] bank with
  accum_out row sums, a tiny f32 mask-matmul that sums each batch's 4
  quarter-sums across partitions, reciprocal, one [128,512] scale, and 4
  output DMAs -- ~6us of tail.

Built as a bacc.Bacc graph (its compile() pass redistributes multi-sem
waits; raw Bass hits the hardware's one-sync-wait-per-instruction limit).
"""

import sys

if "/opt/trn_rl_repo" not in sys.path:
    sys.path.insert(0, "/opt/trn_rl_repo")

import numpy as np

B, H, N = 128, 256, 2048
NCORES = 8
BPC = B // NCORES  # batches per core
P = 128            # SBUF partitions
KT = 4             # k-tiles over 2H=512 contraction
MT = 2             # m-tiles over H=256 output rows
NS = 512           # n-tile (one PSUM bank of f32)
NT = N // NS       # 4 n-tiles
PREFETCH = 8       # batches of x in flight (bf16 x halves SBUF per batch)

_CACHE = {}


def _build():
    import concourse.bacc as bacc
    from concourse import mybir
    from concourse.tile import TileContext

    f32 = mybir.dt.float32
    bf16 = mybir.dt.bfloat16
    Tanh = mybir.ActivationFunctionType.Tanh
    Exp = mybir.ActivationFunctionType.Exp

    nc = bacc.Bacc()
    # xr[b, p, j, n]: host-repacked activations, TRUNCATED to bf16 on the
    # host (same numerics as v1's in-SBUF high-2-byte bitcast, but HALF the
    # HBM traffic: 32MB/core instead of 64MB). j = (xs k0, xs k1, xd k0,
    # xd k1), so each partition's batch slice is ONE 16KB contiguous DRAM
    # run and a j-pair DMA is a single 8KB run per partition.
    xr = nc.declare_dram_parameter("xr", [BPC, P, KT, N], bf16, isOutput=False)
    # wt[k, h] = W[h, k] for k in [0, 512): rows 0:256 static, 256:512 dynamic
    wt = nc.declare_dram_parameter("wt", [2 * H, H], bf16, isOutput=False)
    # cb[h, b] = sum_k W[h, 512+k] * dec[b, k]  (host-precomputed bias)
    cb = nc.declare_dram_parameter("cb", [H, BPC], f32, isOutput=False)
    # vm[p, b, m, j] = v[m*128 + p] * (j == b); columns 16..31 are zero
    vm = nc.declare_dram_parameter("vm", [P, BPC, MT, 32], bf16, isOutput=False)
    # msk[p, j] = ((p%32) == (j%32)) & ((p%32) < 16): partition-sum matrix
    msk = nc.declare_dram_parameter("msk", [P, P], f32, isOutput=False)
    out = nc.declare_dram_parameter("out", [BPC, N], f32, isOutput=True)

    with (
        TileContext(nc) as tc,
        tc.tile_pool(name="const", bufs=1) as cpool,
        tc.tile_pool(name="xh", bufs=PREFETCH) as hpool,
        tc.tile_pool(name="sc", bufs=2) as spool,
        tc.tile_pool(name="ps", bufs=6, space="PSUM") as ppool,
        tc.tile_pool(name="pl", bufs=1, space="PSUM") as plpool,
        tc.tile_pool(name="pq", bufs=1, space="PSUM") as pqpool,
    ):
        # --- x loads: one [128, 4, 2048] bf16 tile per batch, filled by two
        # 1MB HWDGE DMAs (j-pairs; one 8KB contiguous DRAM run per
        # partition => 128 cheap descriptors each). Multi-run-per-partition
        # APs cost ~4x more descriptor-gen, so j is the only safe split
        # axis for steady batches; ramp batches split j singly (4KB runs)
        # plus by columns (j-extent 1 keeps it one run per partition).
        xf_tiles = {}

        def issue_x_dmas(bb, jsplit=2, csplit=1):
            xt = hpool.tile([P, KT, N], bf16, name=f"xt{bb}", tag="xt")
            step = N // csplit
            for cs0 in range(csplit):
                cs = slice(cs0 * step, (cs0 + 1) * step)
                for j0 in range(0, KT, jsplit):
                    js = slice(j0, j0 + jsplit)
                    nc.sync.dma_start(
                        out=xt[:, js, cs],
                        in_=xr[bb, :, js, cs],
                    )
            xf_tiles[bb] = xt

        # --- replicated parameters: wt goes FIRST on the sync (HWDGE) ring
        # so the first matmul's weights land before batch 0's x; the rest
        # ride the idle GPSIMD (SWDGE) ring.
        wt_sb = []
        for kt in range(KT):
            w = cpool.tile([P, H], bf16, name=f"wt{kt}", tag=f"wt{kt}")
            nc.sync.dma_start(out=w[:], in_=wt[kt * P:(kt + 1) * P, :])
            wt_sb.append(w)
        vm_sb = cpool.tile([P, BPC, MT, 32], bf16)
        nc.gpsimd.dma_start(out=vm_sb[:], in_=vm[:])
        # bias laid out [128, m, b]
        c_sb = cpool.tile([P, MT, BPC], f32)
        nc.gpsimd.dma_start(out=c_sb[:], in_=cb[:].rearrange("(m p) b -> p m b", p=P))
        msk_sb = cpool.tile([P, P], f32)
        nc.gpsimd.dma_start(out=msk_sb[:], in_=msk[:])

        issue_x_dmas(0, jsplit=1, csplit=2)
        issue_x_dmas(1, jsplit=1)
        for bb in range(2, PREFETCH):
            issue_x_dmas(bb)

        # logits accumulator: ONE PSUM bank. The masked v-matmul for
        # (batch b, n-tile nt) lands batch b's 512 logits on partition
        # 32*nt + b, accumulating all 16 batches x 2 m-halves per quarter.
        lp = plpool.tile([P, NS], f32)

        sc_hist = {}

        def emit_vmms(vb):
            sc_prev = sc_hist.pop(vb)
            for m in range(MT):
                for nt in range(NT):
                    nc.tensor.matmul(
                        lp[32 * nt:32 * nt + 32, :],
                        lhsT=vm_sb[:, vb, m, :],
                        rhs=sc_prev[:, m, nt * NS:(nt + 1) * NS],
                        start=(vb == 0 and m == 0),
                        stop=(vb == BPC - 1 and m == MT - 1),
                        tile_position=(0, 32 * nt),
                    )

        # --- main loop ---
        for b in range(BPC):
            if b + PREFETCH < BPC:
                issue_x_dmas(b + PREFETCH)
            xt = xf_tiles.pop(b)
            xh = [xt[:, kt, :] for kt in range(KT)]

            # nt-outer / kt-inner: each (nt, m) PSUM unit's 4 accumulating
            # matmuls run consecutively and its tanh follows immediately,
            # so slot releases (and the sc columns the v-matmuls need) are
            # produced evenly through the batch instead of bunching at
            # m-group ends.
            sc_t = spool.tile([P, MT, N], bf16, tag="sc")
            for nt in range(NT):
                ns = slice(nt * NS, (nt + 1) * NS)
                for m in range(MT):
                    pst = ppool.tile([P, NS], f32, tag="pst", name=f"pst{m}_{nt}")
                    for kt in range(KT):
                        nc.tensor.matmul(
                            pst[:],
                            lhsT=wt_sb[kt][:, m * P:(m + 1) * P],
                            rhs=xh[kt][:, ns],
                            start=(kt == 0),
                            stop=(kt == KT - 1),
                        )
                    nc.scalar.activation(
                        sc_t[:, m, ns], pst[:], Tanh,
                        bias=c_sb[:, m, b:b + 1],
                    )
                    if b == BPC - 1:
                        # last batch: emit each v-matmul right after its
                        # tanh so only ONE vmm (not 8) sits between the
                        # final tanh and the softmax chain (~1.6us tail cut)
                        nc.tensor.matmul(
                            lp[32 * nt:32 * nt + 32, :],
                            lhsT=vm_sb[:, b, m, :],
                            rhs=sc_t[:, m, ns],
                            start=False,
                            stop=(m == MT - 1),
                            tile_position=(0, 32 * nt),
                        )
                    if b > 0 and nt == 0 and (
                        m == (0 if b == BPC - 1 else MT - 1)
                    ):
                        # v-matmuls of the previous batch go AFTER batch
                        # b's first 1-2 PSUM units: by now the previous
                        # batch's LAST tanh (produced ~0.8us after its
                        # mains finished) is complete, so all 8 v-matmuls
                        # have ready inputs and the col-tiled groups stream
                        # concurrently instead of stalling the PE on the
                        # tanh tail (measured ~290ns/batch at batch-head
                        # placement). For the last batch the group must
                        # stay BEFORE the first inline stop=True v-matmul
                        # (write-after-stop on the lp accumulation groups),
                        # i.e. after unit (nt0, m0) rather than (nt0, m1).
                        emit_vmms(b - 1)
            if b < BPC - 1:
                sc_hist[b] = sc_t

        # --- softmax over N per batch row (no max-subtraction: |logits| <~ 10)
        # exp of the whole logits bank at once; row sums via accum_out.
        exp_sb = cpool.tile([P, NS], f32)
        psums = cpool.tile([P, 1], f32)
        nc.scalar.activation(exp_sb[:], lp[:], Exp, accum_out=psums[:])
        # sum each batch's 4 quarter-sums across partitions: lpsum[32nt+b]
        # = sum_nt' psums[32nt'+b] for all 4 nt (f32 matmul, 1 column).
        lpsum = pqpool.tile([P, 1], f32)
        nc.tensor.matmul(lpsum[:], lhsT=msk_sb[:], rhs=psums[:], start=True, stop=True)
        rec = cpool.tile([P, 1], f32)
        nc.vector.reciprocal(rec[:], lpsum[:])
        nc.vector.tensor_scalar_mul(exp_sb[:], exp_sb[:], rec[:])
        for nt in range(NT):
            nc.sync.dma_start(
                out=out[:, nt * NS:(nt + 1) * NS],
                in_=exp_sb[32 * nt:32 * nt + BPC, :],
            )

    nc.compile()
    return nc


def _make_in_maps(static_hidden, dynamic_hidden, decoder_hidden, v, W):
    import ml_dtypes

    bf16 = ml_dtypes.bfloat16
    W0 = np.asarray(W, dtype=np.float32)[0]          # [256, 768]
    wt_np = np.ascontiguousarray(W0[:, :2 * H].T.astype(bf16))   # [512, 256]
    vhalf = np.asarray(v, dtype=np.float32)[0, 0].reshape(MT, P)  # [2, 128]
    # vm[p, b, m, j] = v[m*128+p] * (j == b); j in [0, 32), cols 16..31 zero
    eye = np.zeros((BPC, 32), dtype=np.float32)
    eye[np.arange(BPC), np.arange(BPC)] = 1.0
    vm_np = np.ascontiguousarray(
        np.einsum("mp,bj->pbmj", vhalf, eye).astype(bf16)
    )
    # msk[p, j] = ((p%32) == (j%32)) & ((p%32) < 16)
    pp = np.arange(P)
    msk_np = np.ascontiguousarray(
        (((pp[:, None] % 32) == (pp[None, :] % 32)) & ((pp[:, None] % 32) < 16))
        .astype(np.float32)
    )

    sh = np.asarray(static_hidden, dtype=np.float32)
    dh = np.asarray(dynamic_hidden, dtype=np.float32)
    # Truncate f32 -> bf16 on the host (keep the high 2 bytes of each f32;
    # little-endian so uint16 index 1). Identical numerics to v1's in-SBUF
    # stride-2 bitcast, but the DMA moves half the bytes.
    shu = sh.view(np.uint16).reshape(B, 2, P, N, 2)[..., 1]
    dhu = dh.view(np.uint16).reshape(B, 2, P, N, 2)[..., 1]
    # xr[b, p, j, n], j = (xs k0, xs k1, xd k0, xd k1): layout repack so
    # each partition's batch slice is one contiguous 16KB DRAM run
    xr_full = np.concatenate(
        (shu.transpose(0, 2, 1, 3), dhu.transpose(0, 2, 1, 3)), axis=2
    ).view(bf16)                                     # [B, 128, 4, 2048] bf16
    dec = np.asarray(decoder_hidden, dtype=np.float32)
    # cb[h, b] = sum_k W_dec[h, k] dec[b, k], fp32 on host (tiny)
    cb_full = W0[:, 2 * H:] @ dec.T                  # [256, B]

    in_maps = []
    for i in range(NCORES):
        sl = slice(i * BPC, (i + 1) * BPC)
        in_maps.append({
            "xr": np.ascontiguousarray(xr_full[sl]),
            "wt": wt_np,
            "cb": np.ascontiguousarray(cb_full[:, sl]),
            "vm": vm_np,
            "msk": msk_np,
        })
    return in_maps


def kernel(static_hidden, dynamic_hidden, decoder_hidden, v, W):
    from concourse.bass_utils import run_bass_kernel_spmd

    if "nc" not in _CACHE:
        _CACHE["nc"] = _build()
    nc = _CACHE["nc"]

    in_maps = _make_in_maps(static_hidden, dynamic_hidden, decoder_hidden, v, W)
    res = run_bass_kernel_spmd(nc, in_maps, core_ids=list(range(NCORES)))
    out = np.concatenate([r["out"] for r in res.results], axis=0)
    return out.reshape(B, 1, N).astype(np.float32)

